# revision 11
# baseline (speedup 1.0000x reference)
"""Trainium2 Bass kernel for nn_CausalAttention_84018150244353 (v2).

kernel(**inputs) takes the FULL unsharded inputs (as in reference
setup_inputs) and returns the full (2, 2048, 2048) float32 output.

Sharding: 8 NeuronCores = 2 batches x 4 head-groups (4 heads each).
Per core: QKV projections (bf16 matmuls, outputs SBUF-resident),
causal ALiBi attention in transposed layout (keys on partitions,
queries on free), inlined partial output projection y^T = Wo_s^T@out^T;
host sums the 4 head-group partials per batch and adds bo.

Design (vs the original f32r kernel):
 - bf16 operands everywhere (PE rate identical, half DMA/SBUF traffic)
 - q/k/v and Wo SBUF-resident; no DRAM round-trip between phases
 - mshift (ALiBi max-shift) / causal-mask adds on DVE, not PE matmuls
 - softmax denominator: GpSimd-accumulated (c<3) with the reduce/rcp/mul
   deferred one head-block to hide the chain tail; PE-accumulated for
   the largest block (c==3) where filler is scarce
 - q/k bias folded into the PSUM->SBUF activation copy (Identity+bias);
   v/o bias folded into the host epilogue (bo + bv @ Wo)
 - diagonal score tiles trimmed to the unmasked query range
 - fine-grained filler: Q1-Q3 projection tiles and Y output-projection
   tiles are pumped into attention blocks so the PE never starves on
   the DVE/Act/GpSimd softmax chain
 - x quarter 0 + first wq chunk software-pipelined across reps
   (reloaded mid-body), yT stored as bf16, timing loop unrolled x4
"""
import math
import os
import sys
import time

sys.path.insert(0, "/opt/trn_rl_repo")

import numpy as np
import jax

jax.config.update("jax_compilation_cache_dir",
                  os.environ.get("JAX_NEFF_CACHE", "/tmp/jax_neff_cache"))
jax.config.update("jax_persistent_cache_min_compile_time_secs", 0.0)
jax.config.update("jax_persistent_cache_min_entry_size_bytes", -1)

from jax.sharding import Mesh, PartitionSpec
from jax.experimental.shard_map import shard_map

import concourse.bass as bass
import concourse.mybir as mybir
import concourse.tile as tile
from concourse import bacc
from concourse import bass2jax
from concourse.alu_op_type import AluOpType
from concourse.bass2jax import _bass_exec_p, install_neuronx_cc_hook

f32 = mybir.dt.float32
r32 = mybir.dt.float32r
bf16 = mybir.dt.bfloat16
Exp = mybir.ActivationFunctionType.Exp
Identity = mybir.ActivationFunctionType.Identity

T = 2048
EMB = 2048
HG = 512          # columns per head group (4 heads x 128)
HD = 128
NH = 4            # heads per core
NQ = 4            # T quarters
QT = T // NQ      # 512
NE = EMB // 128   # 16 contraction chunks
NJ = T // 128     # 16 key chunks


def build_program(reps: int = 1):
    nc = bacc.Bacc("TRN2", target_bir_lowering=False, debug=False,
                   enable_asserts=False, num_devices=8)

    xq_d = nc.dram_tensor("xq", [128, NQ, NE, QT], bf16, kind="ExternalInput")
    wq_d = nc.dram_tensor("wq", [128, 4, NE, 128], bf16, kind="ExternalInput")
    wk_d = nc.dram_tensor("wk", [128, 4, NE, 128], bf16, kind="ExternalInput")
    wv_d = nc.dram_tensor("wv", [128, NE, HG], bf16, kind="ExternalInput")
    wo_d = nc.dram_tensor("wo", [128, NH, T], bf16, kind="ExternalInput")
    bqk_d = nc.dram_tensor("bqk", [128, 8], f32, kind="ExternalInput")
    alibi_d = nc.dram_tensor("alibi", [128, NH * NJ], f32, kind="ExternalInput")
    mshift_d = nc.dram_tensor("mshift", [1, NH * T], bf16, kind="ExternalInput")
    mask_d = nc.dram_tensor("maskadd", [128, 128], f32, kind="ExternalInput")
    onesb_d = nc.dram_tensor("onesb", [1, 128], bf16, kind="ExternalInput")
    onesr_d = nc.dram_tensor("onesr", [1, 128], r32, kind="ExternalInput")
    yT_d = nc.dram_tensor("yT", [T, T], bf16, kind="ExternalOutput")

    with tile.TileContext(nc) as tc:
        with (
            tc.tile_pool(name="consts", bufs=1) as consts,
            tc.tile_pool(name="qkv", bufs=1) as qkv,
            tc.tile_pool(name="wslab", bufs=1) as wslab,
            tc.tile_pool(name="xslab", bufs=2) as xslab,
            tc.tile_pool(name="xpre", bufs=1) as xpre,
            tc.tile_pool(name="mshp", bufs=4) as mshp,
            tc.tile_pool(name="pp", bufs=8) as pp,
            tc.tile_pool(name="accp", bufs=2) as accp,
            tc.tile_pool(name="rcpp", bufs=2) as rcpp,
            tc.tile_pool(name="ysbp", bufs=3) as ysbp,
            tc.tile_pool(name="ps_s", bufs=5, space="PSUM") as ps_s,
            tc.tile_pool(name="ps_o", bufs=2, space="PSUM") as ps_o,
            tc.tile_pool(name="ps_d", bufs=1, space="PSUM") as ps_d,
        ):
            x_pre = xpre.tile([128, NE, QT], bf16, name="x_pre")
            wq_pre = xpre.tile([128, NE, 128], bf16, name="wq_pre")

            def body(reload_x0=False):
                # persistent SBUF tensors
                qT_sb = qkv.tile([128, NH, T], bf16, name="qT_sb")
                kT_sb = qkv.tile([128, NH, T], bf16, name="kT_sb")
                v_sb = qkv.tile([128, NJ, HG], bf16, name="v_sb")
                wo_sb = qkv.tile([128, NH, T], bf16, name="wo_sb")
                outf = qkv.tile([128, NQ, NH, QT], bf16, name="outf")

                # phase-1 weights
                wq_sb = wslab.tile([128, 4, NE, 128], bf16, name="wq_sb")
                wk_sb = wslab.tile([128, 4, NE, 128], bf16, name="wk_sb")
                wv_sb = wslab.tile([128, NE, HG], bf16, name="wv_sb")

                # consts
                onesb_sb = consts.tile([128, 128], bf16, name="onesb_sb")
                onesr_sb = consts.tile([128, 128], r32, name="onesr_sb")
                alibi_sb = consts.tile([128, NH * NJ], f32, name="alibi_sb")
                mask_sb = consts.tile([128, 128], f32, name="mask_sb")
                bqk_sb = consts.tile([128, 8], f32, name="bqk_sb")

                # DMA order = need order: wq chunks first (x quarter 0
                # is already prefetched in x_pre), then the rest.
                nc.sync.dma_start(wq_sb[:, 1, :, :], wq_d.ap()[:, 1, :, :])
                nc.sync.dma_start(bqk_sb[:], bqk_d.ap())
                nc.sync.dma_start(wq_sb[:, 2, :, :], wq_d.ap()[:, 2, :, :])
                nc.sync.dma_start(wq_sb[:, 3, :, :], wq_d.ap()[:, 3, :, :])
                nc.sync.dma_start(wk_sb[:], wk_d.ap())
                nc.sync.dma_start(wq_sb[:, 0, :, :], wq_d.ap()[:, 0, :, :])
                nc.sync.dma_start(onesb_sb[:],
                                  onesb_d.ap().to_broadcast((128, 128)))
                nc.sync.dma_start(wv_sb[:], wv_d.ap())
                nc.sync.dma_start(onesr_sb[:],
                                  onesr_d.ap().to_broadcast((128, 128)))
                nc.sync.dma_start(alibi_sb[:], alibi_d.ap())
                nc.sync.dma_start(mask_sb[:], mask_d.ap())

                # ---------------- emission helpers ----------------
                def emit_proj_tile(qt, x_sb, k):
                    """One projection tile: k in 0..11 = q cc0-3, k cc0-3,
                    v tb0-3. Shares the ps_s rotation."""
                    ps = ps_s.tile([128, 512], f32, name="psf", tag="ps_s")
                    if k < 8:
                        pi, cc = k // 4, k % 4
                        w_sb = wq_sb if pi == 0 else wk_sb
                        dst = qT_sb if pi == 0 else kT_sb
                        for e in range(NE):
                            w_ap = (wq_pre[:, e, :] if pi == 0 and cc == 0
                                    and x_sb is x_pre
                                    else w_sb[:, cc, e, :])
                            nc.tensor.matmul(
                                ps[:], w_ap, x_sb[:, e, :],
                                start=(e == 0), stop=(e == NE - 1))
                        nc.scalar.activation(
                            dst[:, cc, qt * QT:(qt + 1) * QT], ps[:],
                            Identity,
                            bias=bqk_sb[:, pi * 4 + cc:pi * 4 + cc + 1])
                    else:
                        tb = k - 8
                        for e in range(NE):
                            nc.tensor.matmul(
                                ps[:], x_sb[:, e, tb * 128:(tb + 1) * 128],
                                wv_sb[:, e, :],
                                start=(e == 0), stop=(e == NE - 1))
                        nc.scalar.copy(v_sb[:, qt * 4 + tb, :], ps[:])

                def emit_quarter(qt, x_sb):
                    for k in range(12):
                        emit_proj_tile(qt, x_sb, k)

                def emit_y_tile(c, oc):
                    yp = ps_s.tile([128, 512], f32, name="y_ps", tag="ps_s")
                    for h in range(NH):
                        nc.tensor.matmul(
                            yp[:],
                            wo_sb[:, h, oc * 128:(oc + 1) * 128],
                            outf[:, c, h, :],
                            start=(h == 0), stop=(h == NH - 1))
                    ys = ysbp.tile([128, 512], bf16, name="y_sb", tag="y_sb")
                    nc.scalar.copy(ys[:], yp[:])
                    nc.sync.dma_start(
                        yT_d.ap()[oc * 128:(oc + 1) * 128,
                                  c * QT:(c + 1) * QT],
                        ys[:])

                # filler queue: thunks of independent PE work fed into
                # attention blocks so the PE never starves on the
                # DVE/Act softmax chain
                filler = []

                def pump(n):
                    for _ in range(n):
                        if filler:
                            filler.pop(0)()

                def emit_attn_head(c, h, pump_every=3):
                    msh = mshp.tile([128, 512], bf16, name="msh", tag="msh")
                    nc.sync.dma_start(
                        msh[:],
                        mshift_d.ap()[0:1, h * T + c * QT:h * T + (c + 1) * QT]
                        .to_broadcast((128, 512)))
                    nj = 4 * c + 4
                    s_tiles = {}
                    p_tiles = {}

                    def dstart(jc):
                        return (jc - 4 * c) * 128 if jc >= 4 * c else 0

                    def emit_scores(jc):
                        s = ps_s.tile([128, 512], f32, name="s_ps",
                                      tag="ps_s")
                        ds = dstart(jc)
                        nc.tensor.matmul(
                            s[:, ds:],
                            kT_sb[:, h, jc * 128:(jc + 1) * 128],
                            qT_sb[:, h, c * QT + ds:(c + 1) * QT],
                            start=True, stop=True)
                        nc.vector.tensor_add(s[:, ds:], s[:, ds:],
                                             msh[:, ds:])
                        if jc >= 4 * c:
                            nc.vector.tensor_add(
                                s[:, ds:ds + 128], s[:, ds:ds + 128],
                                mask_sb[:])
                        s_tiles[jc] = s

                    def emit_exp(jc):
                        p = pp.tile([128, 512], bf16, name="p_sb", tag="p_sb")
                        ds = dstart(jc)
                        nc.scalar.activation(
                            p[:, ds:], s_tiles.pop(jc)[:, ds:], Exp,
                            bias=alibi_sb[:, h * NJ + jc:h * NJ + jc + 1])
                        p_tiles[jc] = p

                    outp = ps_o.tile([128, 512], f32, name="out_ps",
                                     tag="out_ps")
                    pe_den = ((c == 3) or (c == 1 and h == 0)
                              or (c == 0 and h == 3))
                    hybrid = (not pe_den) and h == 3
                    diag_ps = []
                    if pe_den:
                        den = ps_d.tile([128, 512], f32, name="den_ps",
                                        tag="den_ps")
                        acc = None
                    else:
                        acc = accp.tile([128, 512], r32, name="acc",
                                        tag="acc")
                        nc.gpsimd.memset(acc[:].bitcast(f32), 0.0)

                    def emit_consume(jc):
                        p = p_tiles.pop(jc)
                        ds = dstart(jc)
                        nc.tensor.matmul(
                            outp[:, ds:],
                            v_sb[:, jc, h * 128:(h + 1) * 128],
                            p[:, ds:],
                            start=(jc == 0), stop=(jc == nj - 1),
                            skip_group_check=True)
                        if pe_den:
                            nc.tensor.matmul(
                                den[:, ds:], onesb_sb[:], p[:, ds:],
                                start=(jc == 0), stop=(jc == nj - 1),
                                skip_group_check=True)
                        elif hybrid and jc >= 4 * c:
                            diag_ps.append((ds, p))
                        else:
                            nc.gpsimd.tensor_add(acc[:, ds:], acc[:, ds:],
                                                 p[:, ds:])

                    LOOK = 3
                    for jc in range(min(LOOK, nj)):
                        emit_scores(jc)
                    for jc in range(nj):
                        if jc + LOOK < nj:
                            emit_scores(jc + LOOK)
                        emit_exp(jc)
                        emit_consume(jc)
                        if jc % pump_every == pump_every - 1:
                            pump(1)

                    def finish():
                        if pe_den:
                            den_t = den
                        else:
                            den_t = ps_d.tile([128, 512], f32, name="den_ps",
                                              tag="den_ps")
                            nc.tensor.matmul(den_t[:], onesr_sb[:], acc[:],
                                             start=True, stop=not diag_ps)
                            for i, (ds, p) in enumerate(diag_ps):
                                nc.tensor.matmul(
                                    den_t[:, ds:], onesb_sb[:], p[:, ds:],
                                    start=False,
                                    stop=(i == len(diag_ps) - 1),
                                    skip_group_check=True)
                        rcp = rcpp.tile([128, 512], f32, name="rcp",
                                        tag="rcp")
                        with nc.allow_low_precision(
                                reason="approx reciprocal"):
                            nc.vector.reciprocal_approx_fast(rcp[:],
                                                             den_t[:])
                            nc.vector.tensor_tensor(
                                outf[:, c, h, :], outp[:], rcp[:],
                                AluOpType.mult)
                    if pe_den:
                        finish()
                        return None
                    return finish

                def emit_y_final(c):
                    # h-major in groups of 4 oc: the first 3 heads'
                    # matmuls run while the last head's outf is pending
                    for g in range(4):
                        yps = []
                        for oc in range(4 * g, 4 * g + 4):
                            yps.append(ps_s.tile([128, 512], f32,
                                                 name="y_ps", tag="ps_s"))
                        for h in range(NH):
                            for i, oc in enumerate(range(4 * g, 4 * g + 4)):
                                nc.tensor.matmul(
                                    yps[i][:],
                                    wo_sb[:, h, oc * 128:(oc + 1) * 128],
                                    outf[:, c, h, :],
                                    start=(h == 0), stop=(h == NH - 1))
                        for i, oc in enumerate(range(4 * g, 4 * g + 4)):
                            ys = ysbp.tile([128, 512], bf16, name="y_sb",
                                           tag="y_sb")
                            nc.scalar.copy(ys[:], yps[i][:])
                            nc.scalar.dma_start(
                                yT_d.ap()[oc * 128:(oc + 1) * 128,
                                          c * QT:(c + 1) * QT],
                                ys[:])

                # ---------------- schedule ----------------
                emit_quarter(0, x_pre)
                x_sb1 = xslab.tile([128, NE, QT], bf16, name="x_sb",
                                   tag="x_sb")
                nc.sync.dma_start(x_sb1[:], xq_d.ap()[:, 1, :, :])
                nc.sync.dma_start(wo_sb[:], wo_d.ap())

                def emit_attn_block(c, pump_every=2):
                    fin = None
                    for h in range(NH):
                        f = emit_attn_head(c, h, pump_every=pump_every)
                        if fin is not None:
                            fin()
                        fin = f
                    if fin is not None:
                        fin()

                # A0 with Q1 tiles as filler (x1 lands during Q0)
                filler.extend(
                    lambda k=k: emit_proj_tile(1, x_sb1, k)
                    for k in range(12))
                emit_attn_block(0)
                pump(99)
                if reload_x0:
                    nc.sync.dma_start(x_pre[:], xq_d.ap()[:, 0, :, :])
                    nc.sync.dma_start(wq_pre[:], wq_d.ap()[:, 0, :, :])

                # A1 with Q2 tiles as filler; x2 DMA issued at A1 start,
                # so hold the Q2 filler back until head 1
                x_sb2 = xslab.tile([128, NE, QT], bf16, name="x_sb",
                                   tag="x_sb")
                nc.sync.dma_start(x_sb2[:], xq_d.ap()[:, 2, :, :])
                fin = emit_attn_head(1, 0)
                filler.extend(
                    lambda k=k: emit_proj_tile(2, x_sb2, k)
                    for k in range(12))
                for h in range(1, NH):
                    f = emit_attn_head(1, h, pump_every=2)
                    if fin is not None:
                        fin()
                    fin = f
                if fin is not None:
                    fin()
                pump(99)

                # A2: Y0 filler first, Q3 filler once x3 has landed
                x_sb3 = xslab.tile([128, NE, QT], bf16, name="x_sb",
                                   tag="x_sb")
                nc.sync.dma_start(x_sb3[:], xq_d.ap()[:, 3, :, :])
                filler.extend(
                    lambda oc=oc: emit_y_tile(0, oc) for oc in range(16))
                f20 = emit_attn_head(2, 0, pump_every=2)
                f21 = emit_attn_head(2, 1, pump_every=2)
                f20()
                filler.extend(
                    lambda k=k: emit_proj_tile(3, x_sb3, k)
                    for k in range(12))
                f22 = emit_attn_head(2, 2, pump_every=2)
                f21()
                f23 = emit_attn_head(2, 3, pump_every=2)
                f22()
                f23()
                pump(99)

                # A3 with Y1, Y2 tiles as filler
                filler.extend(
                    lambda oc=oc: emit_y_tile(1, oc) for oc in range(16))
                emit_attn_head(3, 0, pump_every=2)
                emit_attn_head(3, 1, pump_every=2)
                filler.extend(
                    lambda oc=oc: emit_y_tile(2, oc) for oc in range(16))
                emit_attn_head(3, 2, pump_every=2)
                emit_attn_head(3, 3, pump_every=2)
                pump(99)
                emit_y_final(3)

            nc.sync.dma_start(x_pre[:], xq_d.ap()[:, 0, :, :])
            nc.sync.dma_start(wq_pre[:], wq_d.ap()[:, 0, :, :])
            if reps == 1:
                body()
            elif reps % 8 == 0:
                with tc.For_i(0, reps // 8, 1):
                    for _ in range(8):
                        body(reload_x0=True)
            elif reps % 4 == 0:
                with tc.For_i(0, reps // 4, 1):
                    for _ in range(4):
                        body(reload_x0=True)
            elif reps % 2 == 0:
                with tc.For_i(0, reps // 2, 1):
                    body(reload_x0=True)
                    body(reload_x0=True)
            else:
                with tc.For_i(0, reps, 1):
                    body(reload_x0=True)

    nc.compile()
    return nc


def make_host_inputs(x, Wq, bq, Wk, bk, Wv, bv, Wo, bo):
    """Shard full inputs into 8 per-core input maps."""
    import ml_dtypes
    bfl = ml_dtypes.bfloat16

    x = np.asarray(x, np.float32)
    Wq = np.asarray(Wq, np.float32); bq = np.asarray(bq, np.float32)
    Wk = np.asarray(Wk, np.float32); bk = np.asarray(bk, np.float32)
    Wv = np.asarray(Wv, np.float32); bv = np.asarray(bv, np.float32)
    Wo = np.asarray(Wo, np.float32)

    NUM_HEAD = 16
    start = 2 ** (-2 ** (-(math.log2(NUM_HEAD) - 3)))
    slopes = np.array([start * start ** i for i in range(NUM_HEAD)],
                      np.float32)

    sc = np.float32(1.0 / math.sqrt(HD))
    jl = np.arange(128, dtype=np.float32)
    jcs = np.arange(NJ, dtype=np.float32)
    key_idx = (jcs[None, :] * 128 + jl[:, None])  # [128, NJ]
    i_idx = np.arange(T, dtype=np.float32)

    il = np.arange(128, dtype=np.float32)
    maskadd = np.where(jl[:, None] > il[None, :],
                       np.float32(-1e9), np.float32(0.0))  # [128,128] keys>q

    # x quarter-major: [128, qt, e, t']
    xqs = []
    for b in range(2):
        a = np.ascontiguousarray(x[b].T).astype(bfl)         # [EMB, T]
        a = a.reshape(NE, 128, NQ, QT).transpose(1, 2, 0, 3)  # [p,qt,e,t']
        xqs.append(np.ascontiguousarray(a))

    in_maps = []
    for core in range(8):
        b, hg = core // 4, core % 4
        cols = slice(hg * HG, (hg + 1) * HG)
        heads = slopes[hg * NH:(hg + 1) * NH]
        alibi = np.empty((128, NH * NJ), np.float32)
        mshift = np.empty((1, NH * T), bfl)
        for h in range(NH):
            alibi[:, h * NJ:(h + 1) * NJ] = -heads[h] * (T - 1 - key_idx)
            mshift[0, h * T:(h + 1) * T] = heads[h] * (T - 1 - i_idx)

        def wqk(W, scale):
            a = (W[:, cols] * scale).astype(bfl)             # [EMB, 512]
            a = a.reshape(NE, 128, 4, 128).transpose(1, 2, 0, 3)
            return np.ascontiguousarray(a)                   # [p,cc,e,m]

        wv_a = Wv[:, cols].astype(bfl).reshape(NE, 128, HG)
        wv_a = np.ascontiguousarray(wv_a.transpose(1, 0, 2))  # [p,e,m]
        wo_a = Wo[cols, :].astype(bfl).reshape(NH, 128, T)
        wo_a = np.ascontiguousarray(wo_a.transpose(1, 0, 2))  # [p,h,o]

        bqk = np.empty((128, 8), np.float32)
        bqk[:, 0:4] = (bq[cols] * sc).reshape(4, 128).T
        bqk[:, 4:8] = bk[cols].reshape(4, 128).T

        in_maps.append({
            "xq": xqs[b],
            "wq": wqk(Wq, sc),
            "wk": wqk(Wk, np.float32(1.0)),
            "wv": wv_a,
            "wo": wo_a,
            "bqk": bqk,
            "alibi": alibi,
            "mshift": mshift,
            "maskadd": maskadd,
            "onesb": np.ones((1, 128), bfl),
            "onesr": np.ones((1, 128), np.float32),
        })
    return in_maps


def assemble_output(results, bo, bv, Wo):
    """results: list of 8 per-core dicts with 'yT'. Returns (2, T, EMB)."""
    bo = np.asarray(bo, np.float32)
    boff = bo + np.asarray(bv, np.float32) @ np.asarray(Wo, np.float32)
    out = np.empty((2, T, EMB), np.float32)
    for b in range(2):
        acc = np.asarray(results[b * 4 + 0]["yT"], np.float32).copy()
        for hg in range(1, 4):
            acc += np.asarray(results[b * 4 + hg]["yT"], np.float32)
        out[b] = acc.T + boff
    return out


class SpmdRunner:
    def __init__(self, nc, n_cores: int):
        install_neuronx_cc_hook()
        self.nc = nc
        self.n_cores = n_cores
        assert nc.dbg_addr is None or not nc.dbg_callbacks
        partition_name = (
            nc.partition_id_tensor.name if nc.partition_id_tensor else None
        )
        in_names, out_names, out_avals = [], [], []
        for alloc in nc.m.functions[0].allocations:
            if not isinstance(alloc, mybir.MemoryLocationSet):
                continue
            name = alloc.memorylocations[0].name
            if alloc.kind == "ExternalInput":
                if name != partition_name:
                    in_names.append(name)
            elif alloc.kind == "ExternalOutput":
                shape = tuple(alloc.tensor_shape)
                dtype = mybir.dt.np(alloc.dtype)
                out_names.append(name)
                out_avals.append(jax.core.ShapedArray(shape, dtype))
        self.in_names = list(in_names)
        self.out_names = out_names
        self.out_avals = out_avals
        n_params = len(self.in_names)
        all_in_names = list(in_names) + list(out_names)
        if partition_name is not None:
            all_in_names.append(partition_name)
        self.partition_name = partition_name

        def _body(*args):
            operands = list(args)
            if partition_name is not None:
                operands.append(bass2jax.partition_id_tensor())
            outs = _bass_exec_p.bind(
                *operands,
                out_avals=tuple(out_avals),
                in_names=tuple(all_in_names),
                out_names=tuple(out_names),
                lowering_input_output_aliases=(),
                sim_require_finite=True,
                sim_require_nnan=True,
                nc=nc,
            )
            return tuple(outs)

        devices = jax.devices()[:n_cores]
        assert len(devices) == n_cores
        self.mesh = Mesh(np.asarray(devices), ("core",))
        n_outs = len(out_names)
        in_specs = (PartitionSpec("core"),) * (n_params + n_outs)
        out_specs = (PartitionSpec("core"),) * n_outs
        self.fn = jax.jit(
            shard_map(_body, mesh=self.mesh, in_specs=in_specs,
                      out_specs=out_specs, check_rep=False),
            keep_unused=True,
        )
        self.dev_args = None

    def set_inputs(self, in_maps: list[dict]):
        n = self.n_cores
        assert len(in_maps) == n
        concat_in = [
            np.concatenate([np.asarray(in_maps[c][name]) for c in range(n)],
                           axis=0)
            for name in self.in_names
        ]
        concat_zeros = [
            np.zeros((n * a.shape[0], *a.shape[1:]), a.dtype)
            for a in self.out_avals
        ]
        sharding = jax.sharding.NamedSharding(self.mesh,
                                              PartitionSpec("core"))
        self.dev_args = [jax.device_put(a, sharding)
                         for a in concat_in + concat_zeros]

    def run(self):
        outs = self.fn(*self.dev_args)
        jax.block_until_ready(outs)
        return outs

    def results(self, outs) -> list[dict]:
        n = self.n_cores
        return [
            {
                name: np.asarray(outs[i]).reshape(
                    n, *self.out_avals[i].shape)[c]
                for i, name in enumerate(self.out_names)
            }
            for c in range(n)
        ]

    def time_execs(self, iters: int = 10, warmup: int = 2):
        for _ in range(warmup):
            self.run()
        t0 = time.perf_counter()
        for _ in range(iters):
            outs = self.fn(*self.dev_args)
        jax.block_until_ready(outs)
        t1 = time.perf_counter()
        return (t1 - t0) / iters


_RUNNER = None


def _get_runner():
    global _RUNNER
    if _RUNNER is None:
        nc = build_program(reps=1)
        _RUNNER = SpmdRunner(nc, 8)
    return _RUNNER


def kernel(x, Wq, bq, Wk, bk, Wv, bv, Wo, bo):
    r = _get_runner()
    in_maps = make_host_inputs(x, Wq, bq, Wk, bk, Wv, bv, Wo, bo)
    r.set_inputs(in_maps)
    outs = r.run()
    res = r.results(outs)
    return assemble_output(res, bo, bv, Wo)
